# revision 4
# baseline (speedup 1.0000x reference)
"""Trainium2 Bass kernel for prefix-attention block (B=8,T=1024,C=1024,H=16,Tp=64).

Strategy: data-parallel over batch B across 8 NeuronCores (one batch element
per core, no collectives). Per core, everything is computed in bf16 on the
TensorEngine with f32 PSUM accumulation:

  phase 1: qT,kT in [H*d, T] (head-transposed) layout; v in natural [T, C]
           layout with a per-head ones column appended (so the softmax
           denominator falls out of the AV matmul for free); prefix kpT / vp'
           likewise.
  phase 2: per head, scores are computed transposed  sT[j,i] = k_j . q_i  in
           [128 keys x 512 queries] PSUM tiles (causally trimmed at 128-block
           granularity), exp on ScalarE (scale=1/sqrt(d) folded in), diagonal
           blocks masked by a 0/1 multiply, then the AV matmul accumulates
           unnormalized yT plus the softmax sums (ones column) in PSUM.
           Main and prefix attention keep separate accumulators / sums.
  phase 3: reciprocal of all sums, broadcast across partitions with a tiny
           select-matrix matmul, combine yT = A/sa + B/sb on VectorE, then
           outT = w_proj^T-chunks @ yT. Host transposes the gathered output.
"""

import numpy as np
import ml_dtypes

B, T, C, H, D, TP = 8, 1024, 1024, 16, 64, 64
NT = T // 128   # 8 token tiles
KC = C // 128   # 8 contraction chunks

_CACHE = {}


def _emit(nc, tc, dram):
    import concourse.bass as bass
    import concourse.mybir as mybir
    from contextlib import ExitStack

    BF = mybir.dt.bfloat16
    F32 = mybir.dt.float32
    Exp = mybir.ActivationFunctionType.Exp

    with ExitStack() as top:
        top.enter_context(nc.allow_low_precision(
            reason="bf16 compute is intentional; f32 PSUM accumulation"))
        persist = top.enter_context(tc.tile_pool(name="persist", bufs=1))
        ps_acc = top.enter_context(tc.tile_pool(name="ps_acc", bufs=4, space="PSUM"))
        ps_gen = top.enter_context(tc.tile_pool(name="ps_gen", bufs=4, space="PSUM"))

        qkT = [persist.tile([128, T], BF, tag=f"qkT{m}", name=f"qkT{m}") for m in range(16)]
        vsb = [persist.tile([128, H * 65], BF, tag=f"vsb{t}", name=f"vsb{t}") for t in range(NT)]
        kpT = [persist.tile([128, TP], BF, tag=f"kpT{m}", name=f"kpT{m}") for m in range(8)]
        vpsb = persist.tile([64, H * 65], BF, tag="vpsb", name="vpsb")
        masksb = persist.tile([128, 128], BF, tag="masksb", name="masksb")
        fsb = persist.tile([32, 2 * NT * 128], BF, tag="fsb", name="fsb")
        nc.sync.dma_start(out=masksb, in_=dram["mask"].ap())
        nc.sync.dma_start(out=fsb, in_=dram["fmat"].ap())

        # ---------------- phase 1: projections ----------------
        with ExitStack() as ph1:
            p1 = ph1.enter_context(tc.tile_pool(name="p1", bufs=1))
            pT_t = [p1.tile([128, TP], BF, tag=f"pT{k}", name=f"pT{k}") for k in range(KC)]
            xT_t = [p1.tile([128, T], BF, tag=f"xT{k}", name=f"xT{k}") for k in range(KC)]
            wkp_t = [p1.tile([128, C], BF, tag=f"wkp{k}", name=f"wkp{k}") for k in range(KC)]
            wvp_t = [p1.tile([128, C], BF, tag=f"wvp{k}", name=f"wvp{k}") for k in range(KC)]
            wqk_t = [p1.tile([128, 2 * C], BF, tag=f"wqk{k}", name=f"wqk{k}") for k in range(KC)]
            wv_t = [p1.tile([128, C], BF, tag=f"wv{k}", name=f"wv{k}") for k in range(KC)]
            for k in range(KC):
                r = slice(k * 128, (k + 1) * 128)
                nc.sync.dma_start(out=pT_t[k], in_=dram["pT"].ap()[r, :])
                nc.sync.dma_start(out=xT_t[k], in_=dram["xT"].ap()[r, :])
                nc.sync.dma_start(out=wkp_t[k], in_=dram["wkp"].ap()[r, :])
                nc.sync.dma_start(out=wvp_t[k], in_=dram["wvp"].ap()[r, :])
                nc.sync.dma_start(out=wqk_t[k], in_=dram["wqk"].ap()[r, :])
                nc.sync.dma_start(out=wv_t[k], in_=dram["wv"].ap()[r, :])

            # prefix kT: [128 rows, TP] tiles
            for m in range(8):
                ps = ps_gen.tile([128, TP], F32, tag="ps_g", name="ps_g")
                for k in range(KC):
                    nc.tensor.matmul(ps, wkp_t[k][:, m * 128:(m + 1) * 128],
                                     pT_t[k], start=(k == 0), stop=(k == KC - 1))
                nc.scalar.copy(kpT[m], ps)

            # prefix v' (natural [TP, C] + ones col per head)
            for hf in range(2):
                ps = ps_gen.tile([64, 512], F32, tag="ps_g", name="ps_g")
                for k in range(KC):
                    nc.tensor.matmul(ps, pT_t[k][:, 0:64],
                                     wvp_t[k][:, hf * 512:(hf + 1) * 512],
                                     start=(k == 0), stop=(k == KC - 1))
                for hh in range(8):
                    h = hf * 8 + hh
                    nc.vector.tensor_copy(vpsb[:, h * 65:h * 65 + 64],
                                          ps[:, hh * 64:(hh + 1) * 64])
            for h in range(H):
                nc.vector.memset(vpsb[:, h * 65 + 64:h * 65 + 65], 1.0)

            # q/k transposed: emit q tile then matching k tile so heads
            # unblock early (head pair p needs qkT[p] and qkT[8+p])
            for mm in range(8):
                for m in (mm, 8 + mm):
                    for hf in range(2):
                        ps = ps_gen.tile([128, 512], F32, tag="ps_g", name="ps_g")
                        for k in range(KC):
                            nc.tensor.matmul(
                                ps, wqk_t[k][:, m * 128:(m + 1) * 128],
                                xT_t[k][:, hf * 512:(hf + 1) * 512],
                                start=(k == 0), stop=(k == KC - 1))
                        nc.scalar.copy(qkT[m][:, hf * 512:(hf + 1) * 512], ps)

            # v natural [T, C] + ones cols
            for tt in range(NT):
                for hf in range(2):
                    ps = ps_gen.tile([128, 512], F32, tag="ps_g", name="ps_g")
                    for k in range(KC):
                        nc.tensor.matmul(
                            ps, xT_t[k][:, tt * 128:(tt + 1) * 128],
                            wv_t[k][:, hf * 512:(hf + 1) * 512],
                            start=(k == 0), stop=(k == KC - 1))
                    for hh in range(8):
                        h = hf * 8 + hh
                        nc.vector.tensor_copy(vsb[tt][:, h * 65:h * 65 + 64],
                                              ps[:, hh * 64:(hh + 1) * 64])
                for h in range(H):
                    nc.vector.memset(vsb[tt][:, h * 65 + 64:h * 65 + 65], 1.0)

        # ---------------- phase 2+3: attention ----------------
        with ExitStack() as ph2:
            p2 = ph2.enter_context(tc.tile_pool(name="p2", bufs=1))
            pexp = ph2.enter_context(tc.tile_pool(name="pexp", bufs=6))
            yTa = [p2.tile([128, T], F32, tag=f"yTa{t}", name=f"yTa{t}") for t in range(NT)]
            yTb = [p2.tile([128, T], F32, tag=f"yTb{t}", name=f"yTb{t}") for t in range(NT)]
            yT = [p2.tile([128, T], BF, tag=f"yT{t}", name=f"yT{t}") for t in range(NT)]
            recips = p2.tile([32, T], BF, tag="recips", name="recips")

            for h in range(H):
                pb = (h % 2) * 64          # partition base within tile
                qh = qkT[h // 2]           # rows pb..pb+64 = qT of head h
                kh = qkT[8 + h // 2]
                kph = kpT[h // 2]
                for ir in range(2):
                    i0 = ir * 512
                    # ---- prefix attention (64 keys) ----
                    sp = ps_gen.tile([64, 512], F32, tag="ps_g", name="ps_g")
                    nc.tensor.matmul(sp, kph[pb:pb + 64, :],
                                     qh[pb:pb + 64, i0:i0 + 512],
                                     start=True, stop=True)
                    ep = pexp.tile([64, 512], BF, tag="ep", name="ep")
                    nc.scalar.activation(ep, sp, Exp, scale=0.125)
                    if ir == 0:
                        nc.vector.tensor_mul(ep[:, 0:64], ep[:, 0:64],
                                             masksb[0:64, 0:64])
                    Bt = ps_acc.tile([65, 512], F32, tag="ps_a", name="ps_a")
                    nc.tensor.matmul(Bt, vpsb[:, h * 65:(h + 1) * 65], ep,
                                     start=True, stop=True)
                    # ---- main attention, causally trimmed ----
                    At = ps_acc.tile([65, 512], F32, tag="ps_a", name="ps_a")
                    jmax = 4 * (ir + 1)
                    for jb in range(jmax):
                        c0 = max(0, jb - 4 * ir) * 128
                        st = ps_gen.tile([128, 512], F32, tag="ps_g", name="ps_g")
                        nc.tensor.matmul(st[:, c0:512],
                                         kh[pb:pb + 64, jb * 128:(jb + 1) * 128],
                                         qh[pb:pb + 64, i0 + c0:i0 + 512],
                                         start=True, stop=True)
                        et = pexp.tile([128, 512], BF, tag="et", name="et")
                        nc.scalar.activation(et[:, c0:512], st[:, c0:512],
                                             Exp, scale=0.125)
                        if jb >= 4 * ir:
                            nc.vector.tensor_mul(et[:, c0:c0 + 128],
                                                 et[:, c0:c0 + 128], masksb)
                        nc.tensor.matmul(At[:, c0:512],
                                         vsb[jb][:, h * 65:(h + 1) * 65],
                                         et[:, c0:512],
                                         start=(jb == 0), stop=(jb == jmax - 1),
                                         skip_group_check=True)
                    # ---- stash sums + unnormalized y ----
                    # engine APs must start at partition 0/32/64/96, so the
                    # reciprocal lands in a base-0 scratch row and a tiny DMA
                    # (full partition crossbar) files it into the stack.
                    rca = pexp.tile([1, 512], BF, tag="rc", name="rc")
                    nc.vector.reciprocal(rca, At[64:65, :])
                    nc.sync.dma_start(out=recips[h:h + 1, i0:i0 + 512], in_=rca)
                    rcb = pexp.tile([1, 512], BF, tag="rc", name="rc")
                    nc.vector.reciprocal(rcb, Bt[64:65, :])
                    nc.sync.dma_start(out=recips[16 + h:17 + h, i0:i0 + 512],
                                      in_=rcb)
                    nc.vector.tensor_copy(
                        yTa[h // 2][pb:pb + 64, i0:i0 + 512], At[0:64, :])
                    nc.vector.tensor_copy(
                        yTb[h // 2][pb:pb + 64, i0:i0 + 512], Bt[0:64, :])

            # ---- normalize + combine: yT = A/sa + B/sb ----
            ptmp = ph2.enter_context(tc.tile_pool(name="ptmp", bufs=3))
            for tt in range(NT):
                for hf in range(2):
                    s = slice(hf * 512, (hf + 1) * 512)
                    bca = ps_gen.tile([128, 512], F32, tag="ps_g", name="ps_g")
                    nc.tensor.matmul(bca, fsb[:, tt * 128:(tt + 1) * 128],
                                     recips[:, s], start=True, stop=True)
                    bcb = ps_gen.tile([128, 512], F32, tag="ps_g", name="ps_g")
                    nc.tensor.matmul(bcb, fsb[:, (8 + tt) * 128:(9 + tt) * 128],
                                     recips[:, s], start=True, stop=True)
                    tmp = ptmp.tile([128, 512], BF, tag="tmp", name="tmp")
                    nc.vector.tensor_mul(yT[tt][:, s], yTa[tt][:, s], bca)
                    nc.vector.tensor_mul(tmp, yTb[tt][:, s], bcb)
                    nc.vector.tensor_add(yT[tt][:, s], yT[tt][:, s], tmp)

            # ---- output projection: outT = wp-chunks.T @ yT ----
            with ExitStack() as ph3:
                p3 = ph3.enter_context(tc.tile_pool(name="p3", bufs=1))
                pstg = ph3.enter_context(tc.tile_pool(name="pstg", bufs=3))
                wp_t = [p3.tile([128, C], BF, tag=f"wp{k}", name=f"wp{k}") for k in range(KC)]
                for k in range(KC):
                    nc.sync.dma_start(
                        out=wp_t[k], in_=dram["wp"].ap()[k * 128:(k + 1) * 128, :])
                for m in range(8):
                    stg = pstg.tile([128, T], F32, tag="stg", name="stg")
                    for hf in range(2):
                        po = ps_gen.tile([128, 512], F32, tag="ps_g", name="ps_g")
                        for k in range(KC):
                            nc.tensor.matmul(
                                po, wp_t[k][:, m * 128:(m + 1) * 128],
                                yT[k][:, hf * 512:(hf + 1) * 512],
                                start=(k == 0), stop=(k == KC - 1))
                        nc.scalar.copy(stg[:, hf * 512:(hf + 1) * 512], po)
                    nc.sync.dma_start(
                        out=dram["out"].ap()[m * 128:(m + 1) * 128, :], in_=stg)


def _build():
    if "nc" in _CACHE:
        return _CACHE["nc"]
    import concourse.mybir as mybir
    import concourse.tile as tile
    from concourse import bacc

    BF = mybir.dt.bfloat16
    F32 = mybir.dt.float32
    nc = bacc.Bacc("TRN2", target_bir_lowering=False, debug=False,
                   enable_asserts=False)
    dram = {
        "xT": nc.dram_tensor("xT", [C, T], BF, kind="ExternalInput"),
        "pT": nc.dram_tensor("pT", [C, TP], BF, kind="ExternalInput"),
        "wqk": nc.dram_tensor("wqk", [C, 2 * C], BF, kind="ExternalInput"),
        "wv": nc.dram_tensor("wv", [C, C], BF, kind="ExternalInput"),
        "wkp": nc.dram_tensor("wkp", [C, C], BF, kind="ExternalInput"),
        "wvp": nc.dram_tensor("wvp", [C, C], BF, kind="ExternalInput"),
        "wp": nc.dram_tensor("wp", [C, C], BF, kind="ExternalInput"),
        "mask": nc.dram_tensor("mask", [128, 128], BF, kind="ExternalInput"),
        "fmat": nc.dram_tensor("fmat", [32, 2 * NT * 128], BF,
                               kind="ExternalInput"),
        "out": nc.dram_tensor("out", [C, T], F32, kind="ExternalOutput"),
    }
    with tile.TileContext(nc) as tc:
        _emit(nc, tc, dram)
    nc.compile()
    _CACHE["nc"] = nc
    return nc


def _host_consts():
    bf = ml_dtypes.bfloat16
    mask = np.triu(np.ones((128, 128), np.float32)).astype(bf)  # [p,f]=1 if f>=p
    fmat = np.zeros((32, 2 * NT * 128), np.float32)
    for sel in range(2):          # 0 -> A (rows 0-15), 1 -> B (rows 16-31)
        for tt in range(NT):
            for p in range(128):
                r = sel * 16 + 2 * tt + (1 if p >= 64 else 0)
                fmat[r, (sel * NT + tt) * 128 + p] = 1.0
    return mask, fmat.astype(bf)


def kernel(x, prefix_embd, w_attn, b_attn, w_prefix, b_prefix, w_proj, b_proj,
           **_ignored):
    bf = ml_dtypes.bfloat16
    x = np.asarray(x, np.float32)
    prefix_embd = np.asarray(prefix_embd, np.float32)
    w_attn = np.asarray(w_attn, np.float32)
    w_prefix = np.asarray(w_prefix, np.float32)
    w_proj = np.asarray(w_proj, np.float32)

    nc = _build()
    mask, fmat = _host_consts()
    wqk = np.ascontiguousarray(w_attn[:, :2 * C]).astype(bf)
    wv = np.ascontiguousarray(w_attn[:, 2 * C:]).astype(bf)
    wkp = np.ascontiguousarray(w_prefix[:, C:2 * C]).astype(bf)
    wvp = np.ascontiguousarray(w_prefix[:, 2 * C:]).astype(bf)
    wp = w_proj.astype(bf)
    in_maps = []
    for i in range(B):
        in_maps.append({
            "xT": np.ascontiguousarray(x[i].T).astype(bf),
            "pT": np.ascontiguousarray(prefix_embd[i].T).astype(bf),
            "wqk": wqk, "wv": wv, "wkp": wkp, "wvp": wvp, "wp": wp,
            "mask": mask, "fmat": fmat,
        })

    from concourse.bass_utils import run_bass_kernel_spmd
    res = run_bass_kernel_spmd(nc, in_maps, core_ids=list(range(B)))
    out = np.stack([res.results[i]["out"].T for i in range(B)])
    return np.ascontiguousarray(out.astype(np.float32))


# revision 9
# speedup vs baseline: 1.1083x; 1.1083x over previous
"""Trainium2 Bass kernel for prefix-attention block (B=8,T=1024,C=1024,H=16,Tp=64).

Strategy: data-parallel over batch B across 8 NeuronCores (one batch element
per core, no collectives). Per core, everything is computed in bf16 on the
TensorEngine with f32 PSUM accumulation:

  phase 1: qT,kT in [H*d, T] (head-transposed) layout; v in natural [T, C]
           layout with a per-head ones column appended (so the softmax
           denominator falls out of the AV matmul for free); prefix kpT / vp'
           likewise.
  phase 2: per head, scores are computed transposed  sT[j,i] = k_j . q_i  in
           [128 keys x 512 queries] PSUM tiles (causally trimmed at 128-block
           granularity), exp on ScalarE (scale=1/sqrt(d) folded in), diagonal
           blocks masked by a 0/1 multiply, then the AV matmul accumulates
           unnormalized yT plus the softmax sums (ones column) in PSUM.
           Main and prefix attention keep separate accumulators / sums.
  phase 3: reciprocal of all sums, broadcast across partitions with a tiny
           select-matrix matmul, combine yT = A/sa + B/sb on VectorE, then
           outT = w_proj^T-chunks @ yT. Host transposes the gathered output.
"""

import numpy as np
import ml_dtypes

B, T, C, H, D, TP = 8, 1024, 1024, 16, 64, 64
NT = T // 128   # 8 token tiles
KC = C // 128   # 8 contraction chunks

_CACHE = {}


def _emit(nc, tc, dram):
    import concourse.bass as bass
    import concourse.mybir as mybir
    from contextlib import ExitStack

    BF = mybir.dt.bfloat16
    F32 = mybir.dt.float32
    Exp = mybir.ActivationFunctionType.Exp

    with ExitStack() as top:
        top.enter_context(nc.allow_low_precision(
            reason="bf16 compute is intentional; f32 PSUM accumulation"))
        persist = top.enter_context(tc.tile_pool(name="persist", bufs=1))
        ps_acc = top.enter_context(tc.tile_pool(name="ps_acc", bufs=4, space="PSUM"))
        ps_gen = top.enter_context(tc.tile_pool(name="ps_gen", bufs=4, space="PSUM"))

        qkT = [persist.tile([128, T], BF, tag=f"qkT{m}", name=f"qkT{m}") for m in range(16)]
        vsb = [persist.tile([128, H * 65], BF, tag=f"vsb{t}", name=f"vsb{t}") for t in range(NT)]
        kpT = [persist.tile([128, TP], BF, tag=f"kpT{m}", name=f"kpT{m}") for m in range(8)]
        vpsb = persist.tile([64, H * 65], BF, tag="vpsb", name="vpsb")
        masksb = persist.tile([128, 128], BF, tag="masksb", name="masksb")
        fsb = persist.tile([32, 2 * NT * 128], BF, tag="fsb", name="fsb")
        nc.sync.dma_start(out=masksb, in_=dram["mask"].ap())
        nc.sync.dma_start(out=fsb, in_=dram["fmat"].ap())

        # ---------------- phase 1: projections ----------------
        with ExitStack() as ph1:
            p1 = ph1.enter_context(tc.tile_pool(name="p1", bufs=1))
            pT_t = [p1.tile([128, TP], BF, tag=f"pT{k}", name=f"pT{k}") for k in range(KC)]
            xT_t = [p1.tile([128, T], BF, tag=f"xT{k}", name=f"xT{k}") for k in range(KC)]
            wkp_t = [p1.tile([128, C], BF, tag=f"wkp{k}", name=f"wkp{k}") for k in range(KC)]
            wvp_t = [p1.tile([128, C], BF, tag=f"wvp{k}", name=f"wvp{k}") for k in range(KC)]
            wqk_t = [p1.tile([128, 2 * C], BF, tag=f"wqk{k}", name=f"wqk{k}") for k in range(KC)]
            wv_t = [p1.tile([128, C], BF, tag=f"wv{k}", name=f"wv{k}") for k in range(KC)]
            for k in range(KC):
                r = slice(k * 128, (k + 1) * 128)
                nc.sync.dma_start(out=pT_t[k], in_=dram["pT"].ap()[r, :])
                nc.sync.dma_start(out=xT_t[k], in_=dram["xT"].ap()[r, :])
                nc.sync.dma_start(out=wkp_t[k], in_=dram["wkp"].ap()[r, :])
                nc.sync.dma_start(out=wvp_t[k], in_=dram["wvp"].ap()[r, :])
                nc.sync.dma_start(out=wqk_t[k], in_=dram["wqk"].ap()[r, :])
                nc.sync.dma_start(out=wv_t[k], in_=dram["wv"].ap()[r, :])

            # prefix kT: [128 rows, TP] tiles
            for m in range(8):
                ps = ps_gen.tile([128, TP], F32, tag="ps_g", name="ps_g")
                for k in range(KC):
                    nc.tensor.matmul(ps, wkp_t[k][:, m * 128:(m + 1) * 128],
                                     pT_t[k], start=(k == 0), stop=(k == KC - 1))
                nc.scalar.copy(kpT[m], ps)

            # prefix v' (natural [TP, C] + ones col per head)
            for hf in range(2):
                ps = ps_gen.tile([64, 512], F32, tag="ps_g", name="ps_g")
                for k in range(KC):
                    nc.tensor.matmul(ps, pT_t[k][:, 0:64],
                                     wvp_t[k][:, hf * 512:(hf + 1) * 512],
                                     start=(k == 0), stop=(k == KC - 1))
                vpv = vpsb.rearrange("p (h e) -> p h e", e=65)
                nc.vector.tensor_copy(
                    vpv[:, hf * 8:(hf + 1) * 8, 0:64],
                    ps.rearrange("p (h e) -> p h e", e=64))
            nc.vector.memset(
                vpsb.rearrange("p (h e) -> p h e", e=65)[:, :, 64:65], 1.0)

            # q/k transposed: emit q tile then matching k tile so heads
            # unblock early (head pair p needs qkT[p] and qkT[8+p])
            for mm in range(8):
                for m in (mm, 8 + mm):
                    for hf in range(2):
                        ps = ps_gen.tile([128, 512], F32, tag="ps_g", name="ps_g")
                        for k in range(KC):
                            nc.tensor.matmul(
                                ps, wqk_t[k][:, m * 128:(m + 1) * 128],
                                xT_t[k][:, hf * 512:(hf + 1) * 512],
                                start=(k == 0), stop=(k == KC - 1))
                        nc.scalar.copy(qkT[m][:, hf * 512:(hf + 1) * 512], ps)

            # v natural [T, C] + ones cols
            for tt in range(NT):
                for hf in range(2):
                    ps = ps_gen.tile([128, 512], F32, tag="ps_g", name="ps_g")
                    for k in range(KC):
                        nc.tensor.matmul(
                            ps, xT_t[k][:, tt * 128:(tt + 1) * 128],
                            wv_t[k][:, hf * 512:(hf + 1) * 512],
                            start=(k == 0), stop=(k == KC - 1))
                    nc.vector.tensor_copy(
                        vsb[tt].rearrange("p (h e) -> p h e", e=65)
                        [:, hf * 8:(hf + 1) * 8, 0:64],
                        ps.rearrange("p (h e) -> p h e", e=64))
                nc.vector.memset(
                    vsb[tt].rearrange("p (h e) -> p h e", e=65)[:, :, 64:65],
                    1.0)

        # ---------------- phase 2+3: attention ----------------
        with ExitStack() as ph2:
            p2 = ph2.enter_context(tc.tile_pool(name="p2", bufs=1))
            pexp = ph2.enter_context(tc.tile_pool(name="pexp", bufs=6))
            yTa = [p2.tile([128, T], F32, tag=f"yTa{t}", name=f"yTa{t}") for t in range(NT)]
            yTb = [p2.tile([128, T], F32, tag=f"yTb{t}", name=f"yTb{t}") for t in range(NT)]
            yT = [p2.tile([128, T], BF, tag=f"yT{t}", name=f"yT{t}") for t in range(NT)]
            sums_raw = p2.tile([32, T], F32, tag="sums_raw", name="sums_raw")
            recips = p2.tile([32, T], BF, tag="recips", name="recips")

            for h in range(H):
                pb = (h % 2) * 64          # partition base within tile
                qh = qkT[h // 2]           # rows pb..pb+64 = qT of head h
                kh = qkT[8 + h // 2]
                kph = kpT[h // 2]
                for ir in range(2):
                    i0 = ir * 512
                    # ---- prefix attention (64 keys) ----
                    sp = ps_gen.tile([64, 512], F32, tag="ps_g", name="ps_g")
                    nc.tensor.matmul(sp, kph[pb:pb + 64, :],
                                     qh[pb:pb + 64, i0:i0 + 512],
                                     start=True, stop=True)
                    ep = pexp.tile([64, 512], BF, tag="ep", name="ep")
                    nc.scalar.activation(ep, sp, Exp, scale=0.125)
                    if ir == 0:
                        nc.vector.tensor_mul(ep[:, 0:64], ep[:, 0:64],
                                             masksb[0:64, 0:64])
                    Bt = ps_acc.tile([65, 512], F32, tag="ps_a", name="ps_a")
                    nc.tensor.matmul(Bt, vpsb[:, h * 65:(h + 1) * 65], ep,
                                     start=True, stop=True)
                    # ---- main attention, causally trimmed ----
                    At = ps_acc.tile([65, 512], F32, tag="ps_a", name="ps_a")
                    jmax = 4 * (ir + 1)
                    for jb in range(jmax):
                        c0 = max(0, jb - 4 * ir) * 128
                        st = ps_gen.tile([128, 512], F32, tag="ps_g", name="ps_g")
                        nc.tensor.matmul(st[:, c0:512],
                                         kh[pb:pb + 64, jb * 128:(jb + 1) * 128],
                                         qh[pb:pb + 64, i0 + c0:i0 + 512],
                                         start=True, stop=True)
                        et = pexp.tile([128, 512], BF, tag="et", name="et")
                        nc.scalar.activation(et[:, c0:512], st[:, c0:512],
                                             Exp, scale=0.125)
                        if jb >= 4 * ir:
                            nc.vector.tensor_mul(et[:, c0:c0 + 128],
                                                 et[:, c0:c0 + 128], masksb)
                        nc.tensor.matmul(At[:, c0:512],
                                         vsb[jb][:, h * 65:(h + 1) * 65],
                                         et[:, c0:512],
                                         start=(jb == 0), stop=(jb == jmax - 1),
                                         skip_group_check=True)
                    # ---- stash sums + unnormalized y ----
                    # engine APs must start at partition 0/32/64/96, so the
                    # sum row lands in a base-0 scratch row and a tiny DMA
                    # (full partition crossbar) files it into the stack; one
                    # batched reciprocal runs after all heads.
                    rca = pexp.tile([1, 512], F32, tag="rc", name="rc")
                    nc.vector.tensor_copy(rca, At[64:65, :])
                    nc.sync.dma_start(out=sums_raw[h:h + 1, i0:i0 + 512],
                                      in_=rca)
                    rcb = pexp.tile([1, 512], F32, tag="rc", name="rc")
                    nc.vector.tensor_copy(rcb, Bt[64:65, :])
                    nc.sync.dma_start(out=sums_raw[16 + h:17 + h, i0:i0 + 512],
                                      in_=rcb)
                    nc.vector.tensor_copy(
                        yTa[h // 2][pb:pb + 64, i0:i0 + 512], At[0:64, :])
                    nc.vector.tensor_copy(
                        yTb[h // 2][pb:pb + 64, i0:i0 + 512], Bt[0:64, :])

            # ---- normalize + combine: yT = A/sa + B/sb ----
            nc.vector.reciprocal(recips, sums_raw)
            ptmp = ph2.enter_context(tc.tile_pool(name="ptmp", bufs=3))
            for tt in range(NT):
                for hf in range(2):
                    s = slice(hf * 512, (hf + 1) * 512)
                    bca = ps_gen.tile([128, 512], F32, tag="ps_g", name="ps_g")
                    nc.tensor.matmul(bca, fsb[:, tt * 128:(tt + 1) * 128],
                                     recips[:, s], start=True, stop=True)
                    bcb = ps_gen.tile([128, 512], F32, tag="ps_g", name="ps_g")
                    nc.tensor.matmul(bcb, fsb[:, (8 + tt) * 128:(9 + tt) * 128],
                                     recips[:, s], start=True, stop=True)
                    tmp = ptmp.tile([128, 512], BF, tag="tmp", name="tmp")
                    nc.vector.tensor_mul(yT[tt][:, s], yTa[tt][:, s], bca)
                    nc.vector.tensor_mul(tmp, yTb[tt][:, s], bcb)
                    nc.vector.tensor_add(yT[tt][:, s], yT[tt][:, s], tmp)

            # ---- output projection: outT = wp-chunks.T @ yT ----
            with ExitStack() as ph3:
                p3 = ph3.enter_context(tc.tile_pool(name="p3", bufs=1))
                pstg = ph3.enter_context(tc.tile_pool(name="pstg", bufs=3))
                wp_t = [p3.tile([128, C], BF, tag=f"wp{k}", name=f"wp{k}") for k in range(KC)]
                for k in range(KC):
                    nc.sync.dma_start(
                        out=wp_t[k], in_=dram["wp"].ap()[k * 128:(k + 1) * 128, :])
                for m in range(8):
                    stg = pstg.tile([128, T], F32, tag="stg", name="stg")
                    for hf in range(2):
                        po = ps_gen.tile([128, 512], F32, tag="ps_g", name="ps_g")
                        for k in range(KC):
                            nc.tensor.matmul(
                                po, wp_t[k][:, m * 128:(m + 1) * 128],
                                yT[k][:, hf * 512:(hf + 1) * 512],
                                start=(k == 0), stop=(k == KC - 1))
                        nc.scalar.copy(stg[:, hf * 512:(hf + 1) * 512], po)
                    nc.sync.dma_start(
                        out=dram["out"].ap()[m * 128:(m + 1) * 128, :], in_=stg)


def _build():
    if "nc" in _CACHE:
        return _CACHE["nc"]
    import concourse.mybir as mybir
    import concourse.tile as tile
    from concourse import bacc

    BF = mybir.dt.bfloat16
    F32 = mybir.dt.float32
    nc = bacc.Bacc("TRN2", target_bir_lowering=False, debug=False,
                   enable_asserts=False)
    dram = {
        "xT": nc.dram_tensor("xT", [C, T], BF, kind="ExternalInput"),
        "pT": nc.dram_tensor("pT", [C, TP], BF, kind="ExternalInput"),
        "wqk": nc.dram_tensor("wqk", [C, 2 * C], BF, kind="ExternalInput"),
        "wv": nc.dram_tensor("wv", [C, C], BF, kind="ExternalInput"),
        "wkp": nc.dram_tensor("wkp", [C, C], BF, kind="ExternalInput"),
        "wvp": nc.dram_tensor("wvp", [C, C], BF, kind="ExternalInput"),
        "wp": nc.dram_tensor("wp", [C, C], BF, kind="ExternalInput"),
        "mask": nc.dram_tensor("mask", [128, 128], BF, kind="ExternalInput"),
        "fmat": nc.dram_tensor("fmat", [32, 2 * NT * 128], BF,
                               kind="ExternalInput"),
        "out": nc.dram_tensor("out", [C, T], F32, kind="ExternalOutput"),
    }
    with tile.TileContext(nc) as tc:
        _emit(nc, tc, dram)
    nc.compile()
    _CACHE["nc"] = nc
    return nc


def _host_consts():
    bf = ml_dtypes.bfloat16
    mask = np.triu(np.ones((128, 128), np.float32)).astype(bf)  # [p,f]=1 if f>=p
    fmat = np.zeros((32, 2 * NT * 128), np.float32)
    for sel in range(2):          # 0 -> A (rows 0-15), 1 -> B (rows 16-31)
        for tt in range(NT):
            for p in range(128):
                r = sel * 16 + 2 * tt + (1 if p >= 64 else 0)
                fmat[r, (sel * NT + tt) * 128 + p] = 1.0
    return mask, fmat.astype(bf)


def kernel(x, prefix_embd, w_attn, b_attn, w_prefix, b_prefix, w_proj, b_proj,
           **_ignored):
    bf = ml_dtypes.bfloat16
    x = np.asarray(x, np.float32)
    prefix_embd = np.asarray(prefix_embd, np.float32)
    w_attn = np.asarray(w_attn, np.float32)
    w_prefix = np.asarray(w_prefix, np.float32)
    w_proj = np.asarray(w_proj, np.float32)

    nc = _build()
    mask, fmat = _host_consts()
    wqk = np.ascontiguousarray(w_attn[:, :2 * C]).astype(bf)
    wv = np.ascontiguousarray(w_attn[:, 2 * C:]).astype(bf)
    wkp = np.ascontiguousarray(w_prefix[:, C:2 * C]).astype(bf)
    wvp = np.ascontiguousarray(w_prefix[:, 2 * C:]).astype(bf)
    wp = w_proj.astype(bf)
    in_maps = []
    for i in range(B):
        in_maps.append({
            "xT": np.ascontiguousarray(x[i].T).astype(bf),
            "pT": np.ascontiguousarray(prefix_embd[i].T).astype(bf),
            "wqk": wqk, "wv": wv, "wkp": wkp, "wvp": wvp, "wp": wp,
            "mask": mask, "fmat": fmat,
        })

    from concourse.bass_utils import run_bass_kernel_spmd
    res = run_bass_kernel_spmd(nc, in_maps, core_ids=list(range(B)))
    out = np.stack([res.results[i]["out"].T for i in range(B)])
    return np.ascontiguousarray(out.astype(np.float32))


# revision 15
# speedup vs baseline: 1.1793x; 1.0641x over previous
"""Trainium2 Bass kernel for prefix-attention block (B=8,T=1024,C=1024,H=16,Tp=64).

Strategy: data-parallel over batch B across 8 NeuronCores (one batch element
per core, no collectives). Per core, everything is computed in bf16 on the
TensorEngine with f32 PSUM accumulation:

  phase 1: qT,kT in [H*d, T] (head-transposed) layout; v in natural [T, C]
           layout with a per-head ones column appended (so the softmax
           denominator falls out of the AV matmul for free); prefix kpT / vp'
           likewise.
  phase 2: per head, scores are computed transposed  sT[j,i] = k_j . q_i  in
           [128 keys x 512 queries] PSUM tiles (causally trimmed at 128-block
           granularity), exp on ScalarE (scale=1/sqrt(d) folded in), diagonal
           blocks masked by a 0/1 multiply, then the AV matmul accumulates
           unnormalized yT plus the softmax sums (ones column) in PSUM.
           Main and prefix attention keep separate accumulators / sums.
  phase 3: reciprocal of all sums, broadcast across partitions with a tiny
           select-matrix matmul, combine yT = A/sa + B/sb on VectorE, then
           outT = w_proj^T-chunks @ yT. Host transposes the gathered output.
"""

import numpy as np
import ml_dtypes

B, T, C, H, D, TP = 8, 1024, 1024, 16, 64, 64
NT = T // 128   # 8 token tiles
KC = C // 128   # 8 contraction chunks

_CACHE = {}


def _emit(nc, tc, dram):
    import concourse.bass as bass
    import concourse.mybir as mybir
    from contextlib import ExitStack

    BF = mybir.dt.bfloat16
    F32 = mybir.dt.float32
    Exp = mybir.ActivationFunctionType.Exp

    with ExitStack() as top:
        top.enter_context(nc.allow_low_precision(
            reason="bf16 compute is intentional; f32 PSUM accumulation"))
        persist = top.enter_context(tc.tile_pool(name="persist", bufs=1))
        ps_acc = top.enter_context(tc.tile_pool(name="ps_acc", bufs=4, space="PSUM"))
        ps_gen = top.enter_context(tc.tile_pool(name="ps_gen", bufs=4, space="PSUM"))

        qkT = [persist.tile([128, T], BF, tag=f"qkT{m}", name=f"qkT{m}") for m in range(16)]
        vsb = [persist.tile([128, H * 65], BF, tag=f"vsb{t}", name=f"vsb{t}") for t in range(NT)]
        kpT = [persist.tile([128, TP], BF, tag=f"kpT{m}", name=f"kpT{m}") for m in range(8)]
        vpsb = persist.tile([128, H * 65], BF, tag="vpsb", name="vpsb")
        masksb = persist.tile([128, 128], BF, tag="masksb", name="masksb")
        maskpsb = persist.tile([128, 64], BF, tag="maskpsb", name="maskpsb")
        fsb = persist.tile([128, 2 * NT * 128], BF, tag="fsb", name="fsb")
        nc.sync.dma_start(out=masksb, in_=dram["mask"].ap())
        nc.sync.dma_start(out=maskpsb, in_=dram["maskp"].ap())
        nc.sync.dma_start(out=fsb, in_=dram["fmat"].ap())

        # ---------------- phase 1: projections ----------------
        with ExitStack() as ph1:
            p1 = ph1.enter_context(tc.tile_pool(name="p1", bufs=1))
            pT_t = [p1.tile([128, TP], BF, tag=f"pT{k}", name=f"pT{k}") for k in range(KC)]
            xT_t = [p1.tile([128, T], BF, tag=f"xT{k}", name=f"xT{k}") for k in range(KC)]
            wkp_t = [p1.tile([128, C], BF, tag=f"wkp{k}", name=f"wkp{k}") for k in range(KC)]
            wvp_t = [p1.tile([128, C], BF, tag=f"wvp{k}", name=f"wvp{k}") for k in range(KC)]
            wqk_t = [p1.tile([128, 2 * C], BF, tag=f"wqk{k}", name=f"wqk{k}") for k in range(KC)]
            wv_t = [p1.tile([128, C], BF, tag=f"wv{k}", name=f"wv{k}") for k in range(KC)]
            for k in range(KC):
                r = slice(k * 128, (k + 1) * 128)
                nc.sync.dma_start(out=pT_t[k], in_=dram["pT"].ap()[r, :])
                nc.sync.dma_start(out=xT_t[k], in_=dram["xT"].ap()[r, :])
                nc.sync.dma_start(out=wkp_t[k], in_=dram["wkp"].ap()[r, :])
                nc.sync.dma_start(out=wvp_t[k], in_=dram["wvp"].ap()[r, :])
                nc.sync.dma_start(out=wqk_t[k], in_=dram["wqk"].ap()[r, :])
                nc.sync.dma_start(out=wv_t[k], in_=dram["wv"].ap()[r, :])

            # prefix kT: [128 rows, TP] tiles
            for m in range(8):
                ps = ps_gen.tile([128, TP], F32, tag="ps_g", name="ps_g")
                for k in range(KC):
                    nc.tensor.matmul(ps, wkp_t[k][:, m * 128:(m + 1) * 128],
                                     pT_t[k], start=(k == 0), stop=(k == KC - 1))
                nc.scalar.copy(kpT[m], ps)

            # prefix v' (natural [TP, C] + ones col per head)
            for hf in range(2):
                ps = ps_gen.tile([64, 512], F32, tag="ps_g", name="ps_g")
                for k in range(KC):
                    nc.tensor.matmul(ps, pT_t[k][:, 0:64],
                                     wvp_t[k][:, hf * 512:(hf + 1) * 512],
                                     start=(k == 0), stop=(k == KC - 1))
                vpv = vpsb.rearrange("p (h e) -> p h e", e=65)
                nc.vector.tensor_copy(
                    vpv[0:64, hf * 8:(hf + 1) * 8, 0:64],
                    ps.rearrange("p (h e) -> p h e", e=64))
                nc.vector.tensor_copy(
                    vpv[64:128, hf * 8:(hf + 1) * 8, 0:64],
                    ps.rearrange("p (h e) -> p h e", e=64))
            nc.vector.memset(
                vpsb.rearrange("p (h e) -> p h e", e=65)[:, :, 64:65], 1.0)

            # q/k transposed: emit q tile then matching k tile so heads
            # unblock early (head pair p needs qkT[p] and qkT[8+p])
            for mm in range(8):
                for m in (mm, 8 + mm):
                    for hf in range(2):
                        ps = ps_gen.tile([128, 512], F32, tag="ps_g", name="ps_g")
                        for k in range(KC):
                            nc.tensor.matmul(
                                ps, wqk_t[k][:, m * 128:(m + 1) * 128],
                                xT_t[k][:, hf * 512:(hf + 1) * 512],
                                start=(k == 0), stop=(k == KC - 1))
                        nc.scalar.copy(qkT[m][:, hf * 512:(hf + 1) * 512], ps)

            # v natural [T, C] + ones cols
            for tt in range(NT):
                for hf in range(2):
                    ps = ps_gen.tile([128, 512], F32, tag="ps_g", name="ps_g")
                    for k in range(KC):
                        nc.tensor.matmul(
                            ps, xT_t[k][:, tt * 128:(tt + 1) * 128],
                            wv_t[k][:, hf * 512:(hf + 1) * 512],
                            start=(k == 0), stop=(k == KC - 1))
                    nc.vector.tensor_copy(
                        vsb[tt].rearrange("p (h e) -> p h e", e=65)
                        [:, hf * 8:(hf + 1) * 8, 0:64],
                        ps.rearrange("p (h e) -> p h e", e=64))
                nc.vector.memset(
                    vsb[tt].rearrange("p (h e) -> p h e", e=65)[:, :, 64:65],
                    1.0)

        # ---------------- phase 2+3: attention ----------------
        with ExitStack() as ph2:
            p2 = ph2.enter_context(tc.tile_pool(name="p2", bufs=1))
            pexp = ph2.enter_context(tc.tile_pool(name="pexp", bufs=6))
            yTa = [p2.tile([128, T], F32, tag=f"yTa{t}", name=f"yTa{t}") for t in range(NT)]
            yTb = [p2.tile([128, T], F32, tag=f"yTb{t}", name=f"yTb{t}") for t in range(NT)]
            yT = [p2.tile([128, T], BF, tag=f"yT{t}", name=f"yT{t}") for t in range(NT)]
            sums_raw = p2.tile([32, T], F32, tag="sums_raw", name="sums_raw")
            recips = p2.tile([128, T], BF, tag="recips", name="recips")
            nc.vector.memset(recips, 0.0)

            # Head pairs (even head at partition base 0, odd at 64). The whole
            # phase keeps the PE in a uniform 64-row configuration: scores are
            # K=64 at alternating row bases (concurrent row groups), and each
            # AV matmul is split into two K=64 halves at bases 0/64 that
            # accumulate into the same PSUM tile (row-tiling split-K). This
            # avoids the LDWEIGHTS serialization that K=64 <-> K=128
            # alternation causes.
            for p in range(8):
                qt, kt, kpt = qkT[p], qkT[8 + p], kpT[p]
                for ir in range(2):
                    i0 = ir * 512
                    jmax = 4 * (ir + 1)

                    def scores(jb):
                        c0 = max(0, jb - 4 * ir) * 128
                        ss = []
                        for pb in (0, 64):
                            st = ps_gen.tile([128, 512], F32, tag="ps_g",
                                             name="ps_g")
                            nc.tensor.matmul(
                                st[:, c0:512],
                                kt[pb:pb + 64, jb * 128:(jb + 1) * 128],
                                qt[pb:pb + 64, i0 + c0:i0 + 512],
                                start=True, stop=True)
                            ss.append(st)
                        return ss

                    def exps(jb, ss):
                        c0 = max(0, jb - 4 * ir) * 128
                        es = []
                        for st in ss:
                            et = pexp.tile([128, 512], BF, tag="et", name="et")
                            nc.scalar.activation(et[:, c0:512], st[:, c0:512],
                                                 Exp, scale=0.125)
                            if jb >= 4 * ir:
                                nc.vector.tensor_mul(et[:, c0:c0 + 128],
                                                     et[:, c0:c0 + 128],
                                                     masksb)
                            es.append(et)
                        return es

                    def avs(jb, es, accs):
                        c0 = max(0, jb - 4 * ir) * 128
                        for hh, (et, acc) in enumerate(zip(es, accs)):
                            h = 2 * p + hh
                            nc.tensor.matmul(
                                acc[:, c0:512],
                                vsb[jb][:, h * 65:(h + 1) * 65],
                                et[:, c0:512],
                                start=(jb == 0), stop=(jb == jmax - 1),
                                skip_group_check=True)

                    # prefix: per-head [64,512] banks; K=64 config matches the
                    # score stream (odd head at row base 64 via the duplicated
                    # vp half; avoids tile_position (64,64) = broken quadrant)
                    sps = []
                    for pb in (0, 64):
                        sp = ps_gen.tile([64, 512], F32, tag="ps_g",
                                         name="ps_g")
                        nc.tensor.matmul(sp, kpt[pb:pb + 64, :],
                                         qt[pb:pb + 64, i0:i0 + 512],
                                         start=True, stop=True)
                        sps.append(sp)
                    s_pair = scores(0)
                    eps = []
                    for sp in sps:
                        ep = pexp.tile([64, 512], BF, tag="ep", name="ep")
                        nc.scalar.activation(ep, sp, Exp, scale=0.125)
                        if ir == 0:
                            nc.vector.tensor_mul(ep[:, 0:64], ep[:, 0:64],
                                                 masksb[0:64, 0:64])
                        eps.append(ep)
                    e_pair = exps(0, s_pair)
                    s_next = scores(1)
                    # prefix AV (K=64 at row base 0; ep tiles are at base 0)
                    Bts = [ps_acc.tile([65, 512], F32, tag="ps_a", name="ps_a")
                           for _ in range(2)]
                    for hh in range(2):
                        h = 2 * p + hh
                        nc.tensor.matmul(Bts[hh],
                                         vpsb[0:64, h * 65:(h + 1) * 65],
                                         eps[hh],
                                         start=True, stop=True)
                    Ats = [ps_acc.tile([65, 512], F32, tag="ps_a", name="ps_a")
                           for _ in range(2)]
                    # software pipeline: scores(jb+1) are emitted before
                    # AV(jb) so the PE never waits on the exp chain
                    for jb in range(jmax):
                        e_next = exps(jb + 1, s_next) if jb + 1 < jmax else None
                        if jb + 2 < jmax:
                            s_next = scores(jb + 2)
                        avs(jb, e_pair, Ats)
                        e_pair = e_next
                    # ---- stash sums + unnormalized y ----
                    for hh in range(2):
                        h, pb = 2 * p + hh, hh * 64
                        for acc, row, dst in ((Ats[hh], h, yTa),
                                              (Bts[hh], 16 + h, yTb)):
                            rc = pexp.tile([1, 512], F32, tag="rc", name="rc")
                            nc.scalar.copy(rc, acc[64:65, :])
                            nc.sync.dma_start(
                                out=sums_raw[row:row + 1, i0:i0 + 512], in_=rc)
                            nc.vector.tensor_copy(
                                dst[p][pb:pb + 64, i0:i0 + 512], acc[0:64, :])

            # ---- normalize + combine: yT = A/sa + B/sb ----
            nc.vector.reciprocal(recips[0:32, :], sums_raw)
            ptmp = ph2.enter_context(tc.tile_pool(name="ptmp", bufs=3))
            for tt in range(NT):
                for hf in range(2):
                    s = slice(hf * 512, (hf + 1) * 512)
                    bca = ps_gen.tile([128, 512], F32, tag="ps_g", name="ps_g")
                    nc.tensor.matmul(bca, fsb[:, tt * 128:(tt + 1) * 128],
                                     recips[:, s], start=True, stop=True)
                    bcb = ps_gen.tile([128, 512], F32, tag="ps_g", name="ps_g")
                    nc.tensor.matmul(bcb, fsb[:, (8 + tt) * 128:(9 + tt) * 128],
                                     recips[:, s], start=True, stop=True)
                    tmp = ptmp.tile([128, 512], BF, tag="tmp", name="tmp")
                    nc.vector.tensor_mul(yT[tt][:, s], yTa[tt][:, s], bca)
                    nc.vector.tensor_mul(tmp, yTb[tt][:, s], bcb)
                    nc.vector.tensor_add(yT[tt][:, s], yT[tt][:, s], tmp)

            # ---- output projection: outT = wp-chunks.T @ yT ----
            with ExitStack() as ph3:
                p3 = ph3.enter_context(tc.tile_pool(name="p3", bufs=1))
                pstg = ph3.enter_context(tc.tile_pool(name="pstg", bufs=3))
                wp_t = [p3.tile([128, C], BF, tag=f"wp{k}", name=f"wp{k}") for k in range(KC)]
                for k in range(KC):
                    nc.sync.dma_start(
                        out=wp_t[k], in_=dram["wp"].ap()[k * 128:(k + 1) * 128, :])
                for m in range(8):
                    stg = pstg.tile([128, T], F32, tag="stg", name="stg")
                    for hf in range(2):
                        po = ps_gen.tile([128, 512], F32, tag="ps_g", name="ps_g")
                        for k in range(KC):
                            nc.tensor.matmul(
                                po, wp_t[k][:, m * 128:(m + 1) * 128],
                                yT[k][:, hf * 512:(hf + 1) * 512],
                                start=(k == 0), stop=(k == KC - 1))
                        nc.scalar.copy(stg[:, hf * 512:(hf + 1) * 512], po)
                    nc.sync.dma_start(
                        out=dram["out"].ap()[m * 128:(m + 1) * 128, :], in_=stg)


def _build():
    if "nc" in _CACHE:
        return _CACHE["nc"]
    import concourse.mybir as mybir
    import concourse.tile as tile
    from concourse import bacc

    BF = mybir.dt.bfloat16
    F32 = mybir.dt.float32
    nc = bacc.Bacc("TRN2", target_bir_lowering=False, debug=False,
                   enable_asserts=False)
    dram = {
        "xT": nc.dram_tensor("xT", [C, T], BF, kind="ExternalInput"),
        "pT": nc.dram_tensor("pT", [C, TP], BF, kind="ExternalInput"),
        "wqk": nc.dram_tensor("wqk", [C, 2 * C], BF, kind="ExternalInput"),
        "wv": nc.dram_tensor("wv", [C, C], BF, kind="ExternalInput"),
        "wkp": nc.dram_tensor("wkp", [C, C], BF, kind="ExternalInput"),
        "wvp": nc.dram_tensor("wvp", [C, C], BF, kind="ExternalInput"),
        "wp": nc.dram_tensor("wp", [C, C], BF, kind="ExternalInput"),
        "mask": nc.dram_tensor("mask", [128, 128], BF, kind="ExternalInput"),
        "maskp": nc.dram_tensor("maskp", [128, 64], BF, kind="ExternalInput"),
        "fmat": nc.dram_tensor("fmat", [128, 2 * NT * 128], BF,
                               kind="ExternalInput"),
        "out": nc.dram_tensor("out", [C, T], F32, kind="ExternalOutput"),
    }
    with tile.TileContext(nc) as tc:
        _emit(nc, tc, dram)
    nc.compile()
    _CACHE["nc"] = nc
    return nc


def _host_consts():
    bf = ml_dtypes.bfloat16
    mask = np.triu(np.ones((128, 128), np.float32)).astype(bf)  # [p,f]=1 if f>=p
    tri = np.triu(np.ones((64, 64), np.float32))
    maskp = np.concatenate([tri, tri], axis=0).astype(bf)  # [128, 64]
    fmat = np.zeros((128, 2 * NT * 128), np.float32)
    for sel in range(2):          # 0 -> A (rows 0-15), 1 -> B (rows 16-31)
        for tt in range(NT):
            for p in range(128):
                r = sel * 16 + 2 * tt + (1 if p >= 64 else 0)
                fmat[r, (sel * NT + tt) * 128 + p] = 1.0
    return mask, maskp, fmat.astype(bf)


def _make_in_maps(x, prefix_embd, w_attn, w_prefix, w_proj):
    bf = ml_dtypes.bfloat16
    x = np.asarray(x, np.float32)
    prefix_embd = np.asarray(prefix_embd, np.float32)
    w_attn = np.asarray(w_attn, np.float32)
    w_prefix = np.asarray(w_prefix, np.float32)
    w_proj = np.asarray(w_proj, np.float32)
    mask, maskp, fmat = _host_consts()
    wqk = np.ascontiguousarray(w_attn[:, :2 * C]).astype(bf)
    wv = np.ascontiguousarray(w_attn[:, 2 * C:]).astype(bf)
    wkp = np.ascontiguousarray(w_prefix[:, C:2 * C]).astype(bf)
    wvp = np.ascontiguousarray(w_prefix[:, 2 * C:]).astype(bf)
    wp = w_proj.astype(bf)
    in_maps = []
    for i in range(B):
        in_maps.append({
            "xT": np.ascontiguousarray(x[i].T).astype(bf),
            "pT": np.ascontiguousarray(prefix_embd[i].T).astype(bf),
            "wqk": wqk, "wv": wv, "wkp": wkp, "wvp": wvp, "wp": wp,
            "mask": mask, "maskp": maskp, "fmat": fmat,
        })
    return in_maps


def kernel(x, prefix_embd, w_attn, b_attn, w_prefix, b_prefix, w_proj, b_proj,
           **_ignored):
    nc = _build()
    in_maps = _make_in_maps(x, prefix_embd, w_attn, w_prefix, w_proj)
    from concourse.bass_utils import run_bass_kernel_spmd
    res = run_bass_kernel_spmd(nc, in_maps, core_ids=list(range(B)))
    out = np.stack([res.results[i]["out"].T for i in range(B)])
    return np.ascontiguousarray(out.astype(np.float32))


# revision 18
# speedup vs baseline: 1.2287x; 1.0419x over previous
"""Trainium2 Bass kernel for prefix-attention block (B=8,T=1024,C=1024,H=16,Tp=64).

Strategy: data-parallel over batch B across 8 NeuronCores (one batch element
per core, no collectives). Per core, everything is computed in bf16 on the
TensorEngine with f32 PSUM accumulation:

  phase 1: qT,kT in [H*d, T] (head-transposed) layout; v in natural [T, C]
           layout with a per-head ones column appended (so the softmax
           denominator falls out of the AV matmul for free); prefix kpT / vp'
           likewise.
  phase 2: per head, scores are computed transposed  sT[j,i] = k_j . q_i  in
           [128 keys x 512 queries] PSUM tiles (causally trimmed at 128-block
           granularity), exp on ScalarE (scale=1/sqrt(d) folded in), diagonal
           blocks masked by a 0/1 multiply, then the AV matmul accumulates
           unnormalized yT plus the softmax sums (ones column) in PSUM.
           Main and prefix attention keep separate accumulators / sums.
  phase 3: reciprocal of all sums, broadcast across partitions with a tiny
           select-matrix matmul, combine yT = A/sa + B/sb on VectorE, then
           outT = w_proj^T-chunks @ yT. Host transposes the gathered output.
"""

import numpy as np
import ml_dtypes

B, T, C, H, D, TP = 8, 1024, 1024, 16, 64, 64
NT = T // 128   # 8 token tiles
KC = C // 128   # 8 contraction chunks

_CACHE = {}


def _emit(nc, tc, dram):
    import concourse.bass as bass
    import concourse.mybir as mybir
    from contextlib import ExitStack

    BF = mybir.dt.bfloat16
    F32 = mybir.dt.float32
    Exp = mybir.ActivationFunctionType.Exp

    with ExitStack() as top:
        top.enter_context(nc.allow_low_precision(
            reason="bf16 compute is intentional; f32 PSUM accumulation"))
        persist = top.enter_context(tc.tile_pool(name="persist", bufs=1))
        ps_acc = top.enter_context(tc.tile_pool(name="ps_acc", bufs=3, space="PSUM"))
        ps_gen = top.enter_context(tc.tile_pool(name="ps_gen", bufs=5, space="PSUM"))

        qkT = [persist.tile([128, T], BF, tag=f"qkT{m}", name=f"qkT{m}") for m in range(16)]
        vsb = [persist.tile([128, H * 65], BF, tag=f"vsb{t}", name=f"vsb{t}") for t in range(NT)]
        kpT = [persist.tile([128, TP], BF, tag=f"kpT{m}", name=f"kpT{m}") for m in range(8)]
        vpsb = persist.tile([128, H * 65], BF, tag="vpsb", name="vpsb")
        masksb = persist.tile([128, 128], BF, tag="masksb", name="masksb")
        maskpsb = persist.tile([128, 64], BF, tag="maskpsb", name="maskpsb")
        fsb = persist.tile([128, 2 * NT * 128], BF, tag="fsb", name="fsb")
        nc.sync.dma_start(out=masksb, in_=dram["mask"].ap())
        nc.sync.dma_start(out=maskpsb, in_=dram["maskp"].ap())
        nc.sync.dma_start(out=fsb, in_=dram["fmat"].ap())

        # ---------------- phase 1: projections ----------------
        with ExitStack() as ph1:
            p1 = ph1.enter_context(tc.tile_pool(name="p1", bufs=1))
            pT_t = [p1.tile([128, TP], BF, tag=f"pT{k}", name=f"pT{k}") for k in range(KC)]
            xT_t = [p1.tile([128, T], BF, tag=f"xT{k}", name=f"xT{k}") for k in range(KC)]
            wkp_t = [p1.tile([128, C], BF, tag=f"wkp{k}", name=f"wkp{k}") for k in range(KC)]
            wvp_t = [p1.tile([128, C], BF, tag=f"wvp{k}", name=f"wvp{k}") for k in range(KC)]
            wqk_t = [p1.tile([128, 2 * C], BF, tag=f"wqk{k}", name=f"wqk{k}") for k in range(KC)]
            wv_t = [p1.tile([128, C], BF, tag=f"wv{k}", name=f"wv{k}") for k in range(KC)]
            for k in range(KC):
                r = slice(k * 128, (k + 1) * 128)
                nc.sync.dma_start(out=pT_t[k], in_=dram["pT"].ap()[r, :])
                nc.sync.dma_start(out=xT_t[k], in_=dram["xT"].ap()[r, :])
                nc.sync.dma_start(out=wkp_t[k], in_=dram["wkp"].ap()[r, :])
                nc.sync.dma_start(out=wvp_t[k], in_=dram["wvp"].ap()[r, :])
                nc.sync.dma_start(out=wqk_t[k], in_=dram["wqk"].ap()[r, :])
                nc.sync.dma_start(out=wv_t[k], in_=dram["wv"].ap()[r, :])

            # prefix kT: [128 rows, TP] tiles
            for m in range(8):
                ps = ps_gen.tile([128, TP], F32, tag="ps_g", name="ps_g")
                for k in range(KC):
                    nc.tensor.matmul(ps, wkp_t[k][:, m * 128:(m + 1) * 128],
                                     pT_t[k], start=(k == 0), stop=(k == KC - 1))
                nc.scalar.copy(kpT[m], ps)

            # prefix v' (natural [TP, C] + ones col per head)
            for hf in range(2):
                ps = ps_gen.tile([64, 512], F32, tag="ps_g", name="ps_g")
                for k in range(KC):
                    nc.tensor.matmul(ps, pT_t[k][:, 0:64],
                                     wvp_t[k][:, hf * 512:(hf + 1) * 512],
                                     start=(k == 0), stop=(k == KC - 1))
                vpv = vpsb.rearrange("p (h e) -> p h e", e=65)
                nc.vector.tensor_copy(
                    vpv[0:64, hf * 8:(hf + 1) * 8, 0:64],
                    ps.rearrange("p (h e) -> p h e", e=64))
                nc.vector.tensor_copy(
                    vpv[64:128, hf * 8:(hf + 1) * 8, 0:64],
                    ps.rearrange("p (h e) -> p h e", e=64))
            nc.vector.memset(
                vpsb.rearrange("p (h e) -> p h e", e=65)[:, :, 64:65], 1.0)

            # q/k transposed: emit q tile then matching k tile so heads
            # unblock early (head pair p needs qkT[p] and qkT[8+p])
            for mm in range(8):
                for m in (mm, 8 + mm):
                    for hf in range(2):
                        ps = ps_gen.tile([128, 512], F32, tag="ps_g", name="ps_g")
                        for k in range(KC):
                            nc.tensor.matmul(
                                ps, wqk_t[k][:, m * 128:(m + 1) * 128],
                                xT_t[k][:, hf * 512:(hf + 1) * 512],
                                start=(k == 0), stop=(k == KC - 1))
                        nc.scalar.copy(qkT[m][:, hf * 512:(hf + 1) * 512], ps)

            # v natural [T, C] + ones cols
            for tt in range(NT):
                for hf in range(2):
                    ps = ps_gen.tile([128, 512], F32, tag="ps_g", name="ps_g")
                    for k in range(KC):
                        nc.tensor.matmul(
                            ps, xT_t[k][:, tt * 128:(tt + 1) * 128],
                            wv_t[k][:, hf * 512:(hf + 1) * 512],
                            start=(k == 0), stop=(k == KC - 1))
                    nc.vector.tensor_copy(
                        vsb[tt].rearrange("p (h e) -> p h e", e=65)
                        [:, hf * 8:(hf + 1) * 8, 0:64],
                        ps.rearrange("p (h e) -> p h e", e=64))
                nc.vector.memset(
                    vsb[tt].rearrange("p (h e) -> p h e", e=65)[:, :, 64:65],
                    1.0)

        # ---------------- phase 2+3: attention ----------------
        with ExitStack() as ph2:
            p2 = ph2.enter_context(tc.tile_pool(name="p2", bufs=1))
            pexp = ph2.enter_context(tc.tile_pool(name="pexp", bufs=6))
            yTa = [p2.tile([128, T], F32, tag=f"yTa{t}", name=f"yTa{t}") for t in range(NT)]
            yTb = [p2.tile([128, T], F32, tag=f"yTb{t}", name=f"yTb{t}") for t in range(NT)]
            yT = [p2.tile([128, T], BF, tag=f"yT{t}", name=f"yT{t}") for t in range(NT)]
            sums_raw = p2.tile([32, T], F32, tag="sums_raw", name="sums_raw")
            recips = p2.tile([128, T], BF, tag="recips", name="recips")
            nc.vector.memset(recips, 0.0)

            # Head pairs (even head at partition base 0, odd at 64). Scores
            # are K=64 at alternating row bases (concurrent row groups); AV
            # matmuls are K=128. The PE order is FORCED via dep edges into
            # blocks of [2jb scores][2jb AVs] so the costly 64<->128 row
            # config switch happens once per block instead of every matmul
            # (the Tile scheduler otherwise interleaves them 1:1).
            from concourse.tile_rust import add_dep_helper
            pe_prev = [None]

            def pe_chain(inst):
                if pe_prev[0] is not None:
                    add_dep_helper(inst.ins, pe_prev[0].ins, sync=False,
                                   reason="forced PE order")
                pe_prev[0] = inst

            for p in range(8):
                qt, kt, kpt = qkT[p], qkT[8 + p], kpT[p]
                for ir in range(2):
                    i0 = ir * 512
                    jmax = 4 * (ir + 1)

                    def scores(jb):
                        c0 = max(0, jb - 4 * ir) * 128
                        ss = []
                        for pb in (0, 64):
                            st = ps_gen.tile([128, 512], F32, tag="ps_g",
                                             name="ps_g")
                            pe_chain(nc.tensor.matmul(
                                st[:, c0:512],
                                kt[pb:pb + 64, jb * 128:(jb + 1) * 128],
                                qt[pb:pb + 64, i0 + c0:i0 + 512],
                                start=True, stop=True))
                            ss.append(st)
                        return ss

                    def exps(jb, ss):
                        c0 = max(0, jb - 4 * ir) * 128
                        es = []
                        for st in ss:
                            et = pexp.tile([128, 512], BF, tag="et", name="et")
                            nc.scalar.activation(et[:, c0:512], st[:, c0:512],
                                                 Exp, scale=0.125)
                            if jb >= 4 * ir:
                                nc.vector.tensor_mul(et[:, c0:c0 + 128],
                                                     et[:, c0:c0 + 128],
                                                     masksb)
                            es.append(et)
                        return es

                    def avs(jb, es, accs):
                        c0 = max(0, jb - 4 * ir) * 128
                        for hh, (et, acc) in enumerate(zip(es, accs)):
                            h = 2 * p + hh
                            pe_chain(nc.tensor.matmul(
                                acc[:, c0:512],
                                vsb[jb][:, h * 65:(h + 1) * 65],
                                et[:, c0:512],
                                start=(jb == 0), stop=(jb == jmax - 1),
                                skip_group_check=True))

                    # prefix: per-head [64,512] banks in the K=64 config (odd
                    # head's k/q at row base 64; avoids broken quadrant
                    # tile_position (64,64))
                    sps = []
                    for pb in (0, 64):
                        sp = ps_gen.tile([64, 512], F32, tag="ps_g",
                                         name="ps_g")
                        pe_chain(nc.tensor.matmul(
                            sp, kpt[pb:pb + 64, :],
                            qt[pb:pb + 64, i0:i0 + 512],
                            start=True, stop=True))
                        sps.append(sp)
                    s_all = {0: scores(0), 1: scores(1)}
                    eps = []
                    for sp in sps:
                        ep = pexp.tile([64, 512], BF, tag="ep", name="ep")
                        nc.scalar.activation(ep, sp, Exp, scale=0.125)
                        if ir == 0:
                            nc.vector.tensor_mul(ep[:, 0:64], ep[:, 0:64],
                                                 masksb[0:64, 0:64])
                        eps.append(ep)
                    e_all = {0: exps(0, s_all.pop(0)), 1: exps(1, s_all.pop(1))}
                    # prefix AV (K=64 at row base 0; ep tiles are at base 0)
                    Bts = [ps_acc.tile([65, 512], F32, tag="ps_a", name="ps_a")
                           for _ in range(2)]
                    for hh in range(2):
                        h = 2 * p + hh
                        pe_chain(nc.tensor.matmul(
                            Bts[hh], vpsb[0:64, h * 65:(h + 1) * 65],
                            eps[hh], start=True, stop=True))
                    Ats = [ps_acc.tile([65, 512], F32, tag="ps_a", name="ps_a")
                           for _ in range(2)]
                    # extract B early to free its PSUM slots
                    for hh in range(2):
                        h, pb = 2 * p + hh, hh * 64
                        rc = pexp.tile([1, 512], F32, tag="rc", name="rc")
                        nc.scalar.copy(rc, Bts[hh][64:65, :])
                        nc.sync.dma_start(
                            out=sums_raw[16 + h:17 + h, i0:i0 + 512], in_=rc)
                        nc.vector.tensor_copy(
                            yTb[p][pb:pb + 64, i0:i0 + 512], Bts[hh][0:64, :])
                    # blocks of 2 jb: [AV(jb), AV(jb+1)] then scores of the
                    # next block; exps keep ACT ~1 block ahead
                    for jb0 in range(0, jmax, 2):
                        for jb in (jb0, jb0 + 1):
                            avs(jb, e_all.pop(jb), Ats)
                        for jb in (jb0 + 2, jb0 + 3):
                            if jb < jmax:
                                s_all[jb] = scores(jb)
                        for jb in (jb0 + 2, jb0 + 3):
                            if jb < jmax:
                                e_all[jb] = exps(jb, s_all.pop(jb))
                    # ---- stash sums + unnormalized y ----
                    for hh in range(2):
                        h, pb = 2 * p + hh, hh * 64
                        rc = pexp.tile([1, 512], F32, tag="rc", name="rc")
                        nc.scalar.copy(rc, Ats[hh][64:65, :])
                        nc.sync.dma_start(
                            out=sums_raw[h:h + 1, i0:i0 + 512], in_=rc)
                        nc.vector.tensor_copy(
                            yTa[p][pb:pb + 64, i0:i0 + 512], Ats[hh][0:64, :])

            # ---- normalize + combine: yT = A/sa + B/sb ----
            nc.vector.reciprocal(recips[0:32, :], sums_raw)
            ptmp = ph2.enter_context(tc.tile_pool(name="ptmp", bufs=3))
            for tt in range(NT):
                for hf in range(2):
                    s = slice(hf * 512, (hf + 1) * 512)
                    bca = ps_gen.tile([128, 512], F32, tag="ps_g", name="ps_g")
                    nc.tensor.matmul(bca, fsb[:, tt * 128:(tt + 1) * 128],
                                     recips[:, s], start=True, stop=True)
                    bcb = ps_gen.tile([128, 512], F32, tag="ps_g", name="ps_g")
                    nc.tensor.matmul(bcb, fsb[:, (8 + tt) * 128:(9 + tt) * 128],
                                     recips[:, s], start=True, stop=True)
                    tmp = ptmp.tile([128, 512], BF, tag="tmp", name="tmp")
                    nc.vector.tensor_mul(yT[tt][:, s], yTa[tt][:, s], bca)
                    nc.vector.tensor_mul(tmp, yTb[tt][:, s], bcb)
                    nc.vector.tensor_add(yT[tt][:, s], yT[tt][:, s], tmp)

            # ---- output projection: outT = wp-chunks.T @ yT ----
            with ExitStack() as ph3:
                p3 = ph3.enter_context(tc.tile_pool(name="p3", bufs=1))
                pstg = ph3.enter_context(tc.tile_pool(name="pstg", bufs=3))
                wp_t = [p3.tile([128, C], BF, tag=f"wp{k}", name=f"wp{k}") for k in range(KC)]
                for k in range(KC):
                    nc.sync.dma_start(
                        out=wp_t[k], in_=dram["wp"].ap()[k * 128:(k + 1) * 128, :])
                for m in range(8):
                    stg = pstg.tile([128, T], F32, tag="stg", name="stg")
                    for hf in range(2):
                        po = ps_gen.tile([128, 512], F32, tag="ps_g", name="ps_g")
                        for k in range(KC):
                            nc.tensor.matmul(
                                po, wp_t[k][:, m * 128:(m + 1) * 128],
                                yT[k][:, hf * 512:(hf + 1) * 512],
                                start=(k == 0), stop=(k == KC - 1))
                        nc.scalar.copy(stg[:, hf * 512:(hf + 1) * 512], po)
                    nc.sync.dma_start(
                        out=dram["out"].ap()[m * 128:(m + 1) * 128, :], in_=stg)


def _build():
    if "nc" in _CACHE:
        return _CACHE["nc"]
    import concourse.mybir as mybir
    import concourse.tile as tile
    from concourse import bacc

    BF = mybir.dt.bfloat16
    F32 = mybir.dt.float32
    nc = bacc.Bacc("TRN2", target_bir_lowering=False, debug=False,
                   enable_asserts=False)
    dram = {
        "xT": nc.dram_tensor("xT", [C, T], BF, kind="ExternalInput"),
        "pT": nc.dram_tensor("pT", [C, TP], BF, kind="ExternalInput"),
        "wqk": nc.dram_tensor("wqk", [C, 2 * C], BF, kind="ExternalInput"),
        "wv": nc.dram_tensor("wv", [C, C], BF, kind="ExternalInput"),
        "wkp": nc.dram_tensor("wkp", [C, C], BF, kind="ExternalInput"),
        "wvp": nc.dram_tensor("wvp", [C, C], BF, kind="ExternalInput"),
        "wp": nc.dram_tensor("wp", [C, C], BF, kind="ExternalInput"),
        "mask": nc.dram_tensor("mask", [128, 128], BF, kind="ExternalInput"),
        "maskp": nc.dram_tensor("maskp", [128, 64], BF, kind="ExternalInput"),
        "fmat": nc.dram_tensor("fmat", [128, 2 * NT * 128], BF,
                               kind="ExternalInput"),
        "out": nc.dram_tensor("out", [C, T], F32, kind="ExternalOutput"),
    }
    with tile.TileContext(nc) as tc:
        _emit(nc, tc, dram)
    nc.compile()
    _CACHE["nc"] = nc
    return nc


def _host_consts():
    bf = ml_dtypes.bfloat16
    mask = np.triu(np.ones((128, 128), np.float32)).astype(bf)  # [p,f]=1 if f>=p
    tri = np.triu(np.ones((64, 64), np.float32))
    maskp = np.concatenate([tri, tri], axis=0).astype(bf)  # [128, 64]
    fmat = np.zeros((128, 2 * NT * 128), np.float32)
    for sel in range(2):          # 0 -> A (rows 0-15), 1 -> B (rows 16-31)
        for tt in range(NT):
            for p in range(128):
                r = sel * 16 + 2 * tt + (1 if p >= 64 else 0)
                fmat[r, (sel * NT + tt) * 128 + p] = 1.0
    return mask, maskp, fmat.astype(bf)


def _make_in_maps(x, prefix_embd, w_attn, w_prefix, w_proj):
    bf = ml_dtypes.bfloat16
    x = np.asarray(x, np.float32)
    prefix_embd = np.asarray(prefix_embd, np.float32)
    w_attn = np.asarray(w_attn, np.float32)
    w_prefix = np.asarray(w_prefix, np.float32)
    w_proj = np.asarray(w_proj, np.float32)
    mask, maskp, fmat = _host_consts()
    wqk = np.ascontiguousarray(w_attn[:, :2 * C]).astype(bf)
    wv = np.ascontiguousarray(w_attn[:, 2 * C:]).astype(bf)
    wkp = np.ascontiguousarray(w_prefix[:, C:2 * C]).astype(bf)
    wvp = np.ascontiguousarray(w_prefix[:, 2 * C:]).astype(bf)
    wp = w_proj.astype(bf)
    in_maps = []
    for i in range(B):
        in_maps.append({
            "xT": np.ascontiguousarray(x[i].T).astype(bf),
            "pT": np.ascontiguousarray(prefix_embd[i].T).astype(bf),
            "wqk": wqk, "wv": wv, "wkp": wkp, "wvp": wvp, "wp": wp,
            "mask": mask, "maskp": maskp, "fmat": fmat,
        })
    return in_maps


def kernel(x, prefix_embd, w_attn, b_attn, w_prefix, b_prefix, w_proj, b_proj,
           **_ignored):
    nc = _build()
    in_maps = _make_in_maps(x, prefix_embd, w_attn, w_prefix, w_proj)
    from concourse.bass_utils import run_bass_kernel_spmd
    res = run_bass_kernel_spmd(nc, in_maps, core_ids=list(range(B)))
    out = np.stack([res.results[i]["out"].T for i in range(B)])
    return np.ascontiguousarray(out.astype(np.float32))


# revision 19
# speedup vs baseline: 1.3252x; 1.0785x over previous
"""Trainium2 Bass kernel for prefix-attention block (B=8,T=1024,C=1024,H=16,Tp=64).

Strategy: data-parallel over batch B across 8 NeuronCores (one batch element
per core, no collectives). Per core, everything is computed in bf16 on the
TensorEngine with f32 PSUM accumulation:

  phase 1: qT,kT in [H*d, T] (head-transposed) layout; v in natural [T, C]
           layout with a per-head ones column appended (so the softmax
           denominator falls out of the AV matmul for free); prefix kpT / vp'
           likewise.
  phase 2: per head, scores are computed transposed  sT[j,i] = k_j . q_i  in
           [128 keys x 512 queries] PSUM tiles (causally trimmed at 128-block
           granularity), exp on ScalarE (scale=1/sqrt(d) folded in), diagonal
           blocks masked by a 0/1 multiply, then the AV matmul accumulates
           unnormalized yT plus the softmax sums (ones column) in PSUM.
           Main and prefix attention keep separate accumulators / sums.
  phase 3: reciprocal of all sums, broadcast across partitions with a tiny
           select-matrix matmul, combine yT = A/sa + B/sb on VectorE, then
           outT = w_proj^T-chunks @ yT. Host transposes the gathered output.
"""

import numpy as np
import ml_dtypes

B, T, C, H, D, TP = 8, 1024, 1024, 16, 64, 64
NT = T // 128   # 8 token tiles
KC = C // 128   # 8 contraction chunks

_CACHE = {}


def _emit(nc, tc, dram):
    import concourse.bass as bass
    import concourse.mybir as mybir
    from contextlib import ExitStack

    BF = mybir.dt.bfloat16
    F32 = mybir.dt.float32
    Exp = mybir.ActivationFunctionType.Exp

    with ExitStack() as top:
        top.enter_context(nc.allow_low_precision(
            reason="bf16 compute is intentional; f32 PSUM accumulation"))
        persist = top.enter_context(tc.tile_pool(name="persist", bufs=1))
        ps_acc = top.enter_context(tc.tile_pool(name="ps_acc", bufs=4, space="PSUM"))
        ps_gen = top.enter_context(tc.tile_pool(name="ps_gen", bufs=4, space="PSUM"))

        qkT = [persist.tile([128, T], BF, tag=f"qkT{m}", name=f"qkT{m}") for m in range(16)]
        vsb = [persist.tile([128, H * 65], BF, tag=f"vsb{t}", name=f"vsb{t}") for t in range(NT)]
        kpT = [persist.tile([128, TP], BF, tag=f"kpT{m}", name=f"kpT{m}") for m in range(8)]
        vpsb = persist.tile([128, H * 65], BF, tag="vpsb", name="vpsb")
        masksb = persist.tile([128, 128], BF, tag="masksb", name="masksb")
        maskpsb = persist.tile([128, 64], BF, tag="maskpsb", name="maskpsb")
        fsb = persist.tile([128, 2 * NT * 128], BF, tag="fsb", name="fsb")
        nc.sync.dma_start(out=masksb, in_=dram["mask"].ap())
        nc.sync.dma_start(out=maskpsb, in_=dram["maskp"].ap())
        nc.sync.dma_start(out=fsb, in_=dram["fmat"].ap())

        # ---------------- phase 1: projections ----------------
        with ExitStack() as ph1:
            p1 = ph1.enter_context(tc.tile_pool(name="p1", bufs=1))
            pT_t = [p1.tile([128, TP], BF, tag=f"pT{k}", name=f"pT{k}") for k in range(KC)]
            xT_t = [p1.tile([128, T], BF, tag=f"xT{k}", name=f"xT{k}") for k in range(KC)]
            wkp_t = [p1.tile([128, C], BF, tag=f"wkp{k}", name=f"wkp{k}") for k in range(KC)]
            wvp_t = [p1.tile([128, C], BF, tag=f"wvp{k}", name=f"wvp{k}") for k in range(KC)]
            wqk_t = [p1.tile([128, 2 * C], BF, tag=f"wqk{k}", name=f"wqk{k}") for k in range(KC)]
            wv_t = [p1.tile([128, C], BF, tag=f"wv{k}", name=f"wv{k}") for k in range(KC)]
            for k in range(KC):
                r = slice(k * 128, (k + 1) * 128)
                nc.sync.dma_start(out=pT_t[k], in_=dram["pT"].ap()[r, :])
                nc.sync.dma_start(out=xT_t[k], in_=dram["xT"].ap()[r, :])
                nc.sync.dma_start(out=wkp_t[k], in_=dram["wkp"].ap()[r, :])
                nc.sync.dma_start(out=wvp_t[k], in_=dram["wvp"].ap()[r, :])
                nc.sync.dma_start(out=wqk_t[k], in_=dram["wqk"].ap()[r, :])
                nc.sync.dma_start(out=wv_t[k], in_=dram["wv"].ap()[r, :])

            # prefix kT: [128 rows, TP] tiles
            for m in range(8):
                ps = ps_gen.tile([128, TP], F32, tag="ps_g", name="ps_g")
                for k in range(KC):
                    nc.tensor.matmul(ps, wkp_t[k][:, m * 128:(m + 1) * 128],
                                     pT_t[k], start=(k == 0), stop=(k == KC - 1))
                nc.scalar.copy(kpT[m], ps)

            # prefix v' (natural [TP, C] + ones col per head)
            for hf in range(2):
                ps = ps_gen.tile([64, 512], F32, tag="ps_g", name="ps_g")
                for k in range(KC):
                    nc.tensor.matmul(ps, pT_t[k][:, 0:64],
                                     wvp_t[k][:, hf * 512:(hf + 1) * 512],
                                     start=(k == 0), stop=(k == KC - 1))
                vpv = vpsb.rearrange("p (h e) -> p h e", e=65)
                nc.vector.tensor_copy(
                    vpv[0:64, hf * 8:(hf + 1) * 8, 0:64],
                    ps.rearrange("p (h e) -> p h e", e=64))
                nc.vector.tensor_copy(
                    vpv[64:128, hf * 8:(hf + 1) * 8, 0:64],
                    ps.rearrange("p (h e) -> p h e", e=64))
            nc.vector.memset(
                vpsb.rearrange("p (h e) -> p h e", e=65)[:, :, 64:65], 1.0)

            # q/k transposed: emit q tile then matching k tile so heads
            # unblock early (head pair p needs qkT[p] and qkT[8+p])
            for mm in range(8):
                for m in (mm, 8 + mm):
                    for hf in range(2):
                        ps = ps_gen.tile([128, 512], F32, tag="ps_g", name="ps_g")
                        for k in range(KC):
                            nc.tensor.matmul(
                                ps, wqk_t[k][:, m * 128:(m + 1) * 128],
                                xT_t[k][:, hf * 512:(hf + 1) * 512],
                                start=(k == 0), stop=(k == KC - 1))
                        nc.vector.tensor_copy(qkT[m][:, hf * 512:(hf + 1) * 512], ps)

            # v natural [T, C] + ones cols
            for tt in range(NT):
                for hf in range(2):
                    ps = ps_gen.tile([128, 512], F32, tag="ps_g", name="ps_g")
                    for k in range(KC):
                        nc.tensor.matmul(
                            ps, xT_t[k][:, tt * 128:(tt + 1) * 128],
                            wv_t[k][:, hf * 512:(hf + 1) * 512],
                            start=(k == 0), stop=(k == KC - 1))
                    nc.vector.tensor_copy(
                        vsb[tt].rearrange("p (h e) -> p h e", e=65)
                        [:, hf * 8:(hf + 1) * 8, 0:64],
                        ps.rearrange("p (h e) -> p h e", e=64))
                nc.vector.memset(
                    vsb[tt].rearrange("p (h e) -> p h e", e=65)[:, :, 64:65],
                    1.0)

        # ---------------- phase 2+3: attention ----------------
        with ExitStack() as ph2:
            p2 = ph2.enter_context(tc.tile_pool(name="p2", bufs=1))
            pexp = ph2.enter_context(tc.tile_pool(name="pexp", bufs=6))
            yTa = [p2.tile([128, T], F32, tag=f"yTa{t}", name=f"yTa{t}") for t in range(NT)]
            yTb = [p2.tile([128, T], F32, tag=f"yTb{t}", name=f"yTb{t}") for t in range(NT)]
            yT = [p2.tile([128, T], BF, tag=f"yT{t}", name=f"yT{t}") for t in range(NT)]
            sums_raw = p2.tile([32, T], F32, tag="sums_raw", name="sums_raw")
            recips = p2.tile([128, T], BF, tag="recips", name="recips")
            nc.vector.memset(recips, 0.0)

            # Head pairs (even head at partition base 0, odd at 64). Scores
            # are K=64 at alternating row bases (concurrent row groups); AV
            # matmuls are K=128. The PE order is FORCED via dep edges into
            # blocks of [2jb scores][2jb AVs] so the costly 64<->128 row
            # config switch happens once per block instead of every matmul
            # (the Tile scheduler otherwise interleaves them 1:1).
            from concourse.tile_rust import add_dep_helper
            pe_prev = [None]

            def pe_chain(inst):
                if pe_prev[0] is not None:
                    add_dep_helper(inst.ins, pe_prev[0].ins, sync=False,
                                   reason="forced PE order")
                pe_prev[0] = inst

            for p in range(8):
                qt, kt, kpt = qkT[p], qkT[8 + p], kpT[p]
                for ir in range(2):
                    i0 = ir * 512
                    jmax = 4 * (ir + 1)

                    def scores(jb):
                        c0 = max(0, jb - 4 * ir) * 128
                        ss = []
                        for pb in (0, 64):
                            st = ps_gen.tile([128, 512], F32, tag="ps_g",
                                             name="ps_g")
                            pe_chain(nc.tensor.matmul(
                                st[:, c0:512],
                                kt[pb:pb + 64, jb * 128:(jb + 1) * 128],
                                qt[pb:pb + 64, i0 + c0:i0 + 512],
                                start=True, stop=True))
                            ss.append(st)
                        return ss

                    def exps(jb, ss):
                        c0 = max(0, jb - 4 * ir) * 128
                        es = []
                        for st in ss:
                            et = pexp.tile([128, 512], BF, tag="et", name="et")
                            nc.scalar.activation(et[:, c0:512], st[:, c0:512],
                                                 Exp, scale=0.125)
                            if jb >= 4 * ir:
                                nc.vector.tensor_mul(et[:, c0:c0 + 128],
                                                     et[:, c0:c0 + 128],
                                                     masksb)
                            es.append(et)
                        return es

                    def avs(jb, es, accs):
                        c0 = max(0, jb - 4 * ir) * 128
                        for hh, (et, acc) in enumerate(zip(es, accs)):
                            h = 2 * p + hh
                            pe_chain(nc.tensor.matmul(
                                acc[:, c0:512],
                                vsb[jb][:, h * 65:(h + 1) * 65],
                                et[:, c0:512],
                                start=(jb == 0), stop=(jb == jmax - 1),
                                skip_group_check=True))

                    # prefix: per-head [64,512] banks in the K=64 config (odd
                    # head's k/q at row base 64; avoids broken quadrant
                    # tile_position (64,64))
                    sps = []
                    for pb in (0, 64):
                        sp = ps_gen.tile([64, 512], F32, tag="ps_g",
                                         name="ps_g")
                        pe_chain(nc.tensor.matmul(
                            sp, kpt[pb:pb + 64, :],
                            qt[pb:pb + 64, i0:i0 + 512],
                            start=True, stop=True))
                        sps.append(sp)
                    s_all = {0: scores(0), 1: scores(1)}
                    eps = []
                    for sp in sps:
                        ep = pexp.tile([64, 512], BF, tag="ep", name="ep")
                        nc.scalar.activation(ep, sp, Exp, scale=0.125)
                        if ir == 0:
                            nc.vector.tensor_mul(ep[:, 0:64], ep[:, 0:64],
                                                 masksb[0:64, 0:64])
                        eps.append(ep)
                    e_all = {0: exps(0, s_all.pop(0)), 1: exps(1, s_all.pop(1))}
                    # prefix AV (K=64 at row base 0; ep tiles are at base 0)
                    Bts = [ps_acc.tile([65, 512], F32, tag="ps_a", name="ps_a")
                           for _ in range(2)]
                    for hh in range(2):
                        h = 2 * p + hh
                        pe_chain(nc.tensor.matmul(
                            Bts[hh], vpsb[0:64, h * 65:(h + 1) * 65],
                            eps[hh], start=True, stop=True))
                    Ats = [ps_acc.tile([65, 512], F32, tag="ps_a", name="ps_a")
                           for _ in range(2)]
                    # extract B early to free its PSUM slots
                    for hh in range(2):
                        h, pb = 2 * p + hh, hh * 64
                        rc = pexp.tile([1, 512], F32, tag="rc", name="rc")
                        nc.scalar.copy(rc, Bts[hh][64:65, :])
                        nc.sync.dma_start(
                            out=sums_raw[16 + h:17 + h, i0:i0 + 512], in_=rc)
                        nc.vector.tensor_copy(
                            yTb[p][pb:pb + 64, i0:i0 + 512], Bts[hh][0:64, :])
                    # blocks of 2 jb: [AV(jb), AV(jb+1)] then scores of the
                    # next block; exps keep ACT ~1 block ahead
                    for jb0 in range(0, jmax, 2):
                        for jb in (jb0, jb0 + 1):
                            avs(jb, e_all.pop(jb), Ats)
                        for jb in (jb0 + 2, jb0 + 3):
                            if jb < jmax:
                                s_all[jb] = scores(jb)
                        for jb in (jb0 + 2, jb0 + 3):
                            if jb < jmax:
                                e_all[jb] = exps(jb, s_all.pop(jb))
                    # ---- stash sums + unnormalized y ----
                    for hh in range(2):
                        h, pb = 2 * p + hh, hh * 64
                        rc = pexp.tile([1, 512], F32, tag="rc", name="rc")
                        nc.scalar.copy(rc, Ats[hh][64:65, :])
                        nc.sync.dma_start(
                            out=sums_raw[h:h + 1, i0:i0 + 512], in_=rc)
                        nc.vector.tensor_copy(
                            yTa[p][pb:pb + 64, i0:i0 + 512], Ats[hh][0:64, :])

            # ---- normalize + combine: yT = A/sa + B/sb ----
            nc.vector.reciprocal(recips[0:32, :], sums_raw)
            ptmp = ph2.enter_context(tc.tile_pool(name="ptmp", bufs=3))
            for tt in range(NT):
                for hf in range(2):
                    s = slice(hf * 512, (hf + 1) * 512)
                    bca = ps_gen.tile([128, 512], F32, tag="ps_g", name="ps_g")
                    nc.tensor.matmul(bca, fsb[:, tt * 128:(tt + 1) * 128],
                                     recips[:, s], start=True, stop=True)
                    bcb = ps_gen.tile([128, 512], F32, tag="ps_g", name="ps_g")
                    nc.tensor.matmul(bcb, fsb[:, (8 + tt) * 128:(9 + tt) * 128],
                                     recips[:, s], start=True, stop=True)
                    tmp = ptmp.tile([128, 512], BF, tag="tmp", name="tmp")
                    nc.vector.tensor_mul(yT[tt][:, s], yTa[tt][:, s], bca)
                    nc.vector.tensor_mul(tmp, yTb[tt][:, s], bcb)
                    nc.vector.tensor_add(yT[tt][:, s], yT[tt][:, s], tmp)

            # ---- output projection: outT = wp-chunks.T @ yT ----
            with ExitStack() as ph3:
                p3 = ph3.enter_context(tc.tile_pool(name="p3", bufs=1))
                pstg = ph3.enter_context(tc.tile_pool(name="pstg", bufs=3))
                wp_t = [p3.tile([128, C], BF, tag=f"wp{k}", name=f"wp{k}") for k in range(KC)]
                for k in range(KC):
                    nc.sync.dma_start(
                        out=wp_t[k], in_=dram["wp"].ap()[k * 128:(k + 1) * 128, :])
                for m in range(8):
                    stg = pstg.tile([128, T], F32, tag="stg", name="stg")
                    for hf in range(2):
                        po = ps_gen.tile([128, 512], F32, tag="ps_g", name="ps_g")
                        for k in range(KC):
                            nc.tensor.matmul(
                                po, wp_t[k][:, m * 128:(m + 1) * 128],
                                yT[k][:, hf * 512:(hf + 1) * 512],
                                start=(k == 0), stop=(k == KC - 1))
                        nc.scalar.copy(stg[:, hf * 512:(hf + 1) * 512], po)
                    nc.sync.dma_start(
                        out=dram["out"].ap()[m * 128:(m + 1) * 128, :], in_=stg)


def _build():
    if "nc" in _CACHE:
        return _CACHE["nc"]
    import concourse.mybir as mybir
    import concourse.tile as tile
    from concourse import bacc

    BF = mybir.dt.bfloat16
    F32 = mybir.dt.float32
    nc = bacc.Bacc("TRN2", target_bir_lowering=False, debug=False,
                   enable_asserts=False)
    dram = {
        "xT": nc.dram_tensor("xT", [C, T], BF, kind="ExternalInput"),
        "pT": nc.dram_tensor("pT", [C, TP], BF, kind="ExternalInput"),
        "wqk": nc.dram_tensor("wqk", [C, 2 * C], BF, kind="ExternalInput"),
        "wv": nc.dram_tensor("wv", [C, C], BF, kind="ExternalInput"),
        "wkp": nc.dram_tensor("wkp", [C, C], BF, kind="ExternalInput"),
        "wvp": nc.dram_tensor("wvp", [C, C], BF, kind="ExternalInput"),
        "wp": nc.dram_tensor("wp", [C, C], BF, kind="ExternalInput"),
        "mask": nc.dram_tensor("mask", [128, 128], BF, kind="ExternalInput"),
        "maskp": nc.dram_tensor("maskp", [128, 64], BF, kind="ExternalInput"),
        "fmat": nc.dram_tensor("fmat", [128, 2 * NT * 128], BF,
                               kind="ExternalInput"),
        "out": nc.dram_tensor("out", [C, T], F32, kind="ExternalOutput"),
    }
    with tile.TileContext(nc) as tc:
        _emit(nc, tc, dram)
    nc.compile()
    _CACHE["nc"] = nc
    return nc


def _host_consts():
    bf = ml_dtypes.bfloat16
    mask = np.triu(np.ones((128, 128), np.float32)).astype(bf)  # [p,f]=1 if f>=p
    tri = np.triu(np.ones((64, 64), np.float32))
    maskp = np.concatenate([tri, tri], axis=0).astype(bf)  # [128, 64]
    fmat = np.zeros((128, 2 * NT * 128), np.float32)
    for sel in range(2):          # 0 -> A (rows 0-15), 1 -> B (rows 16-31)
        for tt in range(NT):
            for p in range(128):
                r = sel * 16 + 2 * tt + (1 if p >= 64 else 0)
                fmat[r, (sel * NT + tt) * 128 + p] = 1.0
    return mask, maskp, fmat.astype(bf)


def _make_in_maps(x, prefix_embd, w_attn, w_prefix, w_proj):
    bf = ml_dtypes.bfloat16
    x = np.asarray(x, np.float32)
    prefix_embd = np.asarray(prefix_embd, np.float32)
    w_attn = np.asarray(w_attn, np.float32)
    w_prefix = np.asarray(w_prefix, np.float32)
    w_proj = np.asarray(w_proj, np.float32)
    mask, maskp, fmat = _host_consts()
    wqk = np.ascontiguousarray(w_attn[:, :2 * C]).astype(bf)
    wv = np.ascontiguousarray(w_attn[:, 2 * C:]).astype(bf)
    wkp = np.ascontiguousarray(w_prefix[:, C:2 * C]).astype(bf)
    wvp = np.ascontiguousarray(w_prefix[:, 2 * C:]).astype(bf)
    wp = w_proj.astype(bf)
    in_maps = []
    for i in range(B):
        in_maps.append({
            "xT": np.ascontiguousarray(x[i].T).astype(bf),
            "pT": np.ascontiguousarray(prefix_embd[i].T).astype(bf),
            "wqk": wqk, "wv": wv, "wkp": wkp, "wvp": wvp, "wp": wp,
            "mask": mask, "maskp": maskp, "fmat": fmat,
        })
    return in_maps


def kernel(x, prefix_embd, w_attn, b_attn, w_prefix, b_prefix, w_proj, b_proj,
           **_ignored):
    nc = _build()
    in_maps = _make_in_maps(x, prefix_embd, w_attn, w_prefix, w_proj)
    from concourse.bass_utils import run_bass_kernel_spmd
    res = run_bass_kernel_spmd(nc, in_maps, core_ids=list(range(B)))
    out = np.stack([res.results[i]["out"].T for i in range(B)])
    return np.ascontiguousarray(out.astype(np.float32))
